# revision 40
# baseline (speedup 1.0000x reference)
"""Trainium2 Bass kernel for the KinematicBicycle rollout (H=8192).

kernel(x0, U, dt) -> [8193, 4] float32 trajectory, computed on TRN2.

Algorithm (validated against the jax reference):
  b_t = clip(U[t,0],±3)*dt. The speed recurrence
      w_{t+1} = min(max(w_t + b_t, 0), 30)
  is an alternating clamp chain, computed EXACTLY with the DVE
  tensor_tensor_scan primitive (state = (data0 op0 state) op1 data1) using
  op0=subtract / op1=max over sign-alternating interleaved data:
      even step: state = max(b_t - state, 0)    # state_in = -w_t
      odd  step: state = max(0 - state, -30)    # = -min(., 30) = -w_{t+1}
  Time is chunked [128 partitions x 64 steps]. Chunk-entry speeds come from
  composing per-chunk clamp-add maps F_p(v)=min(max(v+s_p,lo_p),hi_p):
  lo_p/hi_p are obtained by probing each chunk scan with -/+BIG initials, and
  a 2-micro-step composition scan (258 elements on one partition) chains the
  128 maps. Given w, theta is a plain prefix sum of w_t*tan(delta_t)*dt/L and
  x/y are prefix sums of w_t*cos/sin(theta_t)*dt: per-partition
  tensor_tensor_scan + strict-lower-triangular matmul for cross-chunk
  offsets. ACT has only Sin (accurate on ~[-pi,pi]): cos(x)=sin(x+pi/2),
  tan(d)=sin(d)/cos(d); theta is range-reduced with the fp32 magic-constant
  round trick before the table lookups.

The rollout is a single sequential recurrence (no batch dim), so there is
nothing to shard across cores; the program is replicated SPMD on all 8
cores and core 0's output is returned.
"""
import os
import numpy as np

import concourse.bacc as bacc
import concourse.bass as bass
import concourse.mybir as mybir
import concourse.tile as tile
from concourse.bass_utils import run_bass_kernel_spmd

F32 = mybir.dt.float32
OP = mybir.AluOpType
AF = mybir.ActivationFunctionType

H, P, C = 8192, 128, 64
L = 2.7
BIG = 1e30
HPI = float(np.pi / 2)
MAGIC = 12582912.0          # 1.5*2^23: fp32 round-to-nearest via add/sub
INV2PI = float(1.0 / (2.0 * np.pi))
TWOPI = float(2.0 * np.pi)
N_CORES = int(os.environ.get("KB_CORES", "8"))

LAST_RUN_INFO = {}
_CACHE = {}


def _build(dt_val):
    nc = bacc.Bacc("TRN2", target_bir_lowering=False, debug=False)

    dt_f = float(dt_val)

    x0_d = nc.dram_tensor("x0", [4], F32, kind="ExternalInput")
    U_d = nc.dram_tensor("U", [H, 2], F32, kind="ExternalInput")
    out_d = nc.dram_tensor("out", [H + 1, 4], F32, kind="ExternalOutput")

    with tile.TileContext(nc) as tc:
        with (
            tc.tile_pool(name="sb", bufs=1) as sb,
            tc.tile_pool(name="ps", bufs=1, space="PSUM") as ps,
        ):
            # ---- loads; dt is baked into the program as an immediate ----
            Ut = sb.tile([P, 2 * C], F32, tag="Ut")
            nc.sync.dma_start(out=Ut, in_=U_d[:].rearrange("(p j) c -> p (j c)", p=P))
            xrow = sb.tile([1, 8], F32, tag="xrow")
            nc.sync.dma_start(out=xrow[0:1, 0:4],
                              in_=x0_d[:].rearrange("(o a) -> o a", o=1))

            hpi_b = sb.tile([P, 1], F32, tag="hpi_b")
            nc.gpsimd.memset(hpi_b, HPI)
            zero_b = sb.tile([P, 1], F32, tag="zero_b")
            nc.gpsimd.memset(zero_b, 0.0)
            d0v = sb.tile([P, 2 * C], F32, tag="d0v")
            nc.gpsimd.memset(d0v, 0.0)
            d1v = sb.tile([P, 2 * C], F32, tag="d1v")
            nc.gpsimd.memset(d1v, 0.0)
            nc.gpsimd.memset(d1v[:, 1:2 * C:2], -30.0)

            # First ACT op depends only on the early memsets, so the 1.3us
            # ACT table load (inserted before the first ACTIVATE and after
            # its semaphore wait) runs during the DMA window.
            warm = sb.tile([P, 1], F32, tag="warm")
            nc.scalar.activation(warm, hpi_b, AF.Sin, bias=zero_b)

            # ---- controls ----
            dcl = sb.tile([P, C], F32, tag="dcl")
            nc.vector.tensor_scalar(dcl, Ut[:, 1:2 * C:2], -0.6, 0.6, OP.max, OP.min)
            sin_d = sb.tile([P, C], F32, tag="sin_d")
            nc.scalar.activation(sin_d, dcl, AF.Sin, bias=zero_b)
            cos_d = sb.tile([P, C], F32, tag="cos_d")
            nc.scalar.activation(cos_d, dcl, AF.Sin, bias=hpi_b)
            bcl = sb.tile([P, C], F32, tag="bcl")
            nc.vector.tensor_scalar(bcl, Ut[:, 0:2 * C:2], -3.0, 3.0, OP.max, OP.min)
            # write b = bcl*dt straight into the even slots of the scan input
            nc.vector.tensor_scalar_mul(d0v[:, 0:2 * C:2], bcl, dt_f)
            # ---- v-scan pass 1: per-chunk probes; batch-transpose maps ----
            slo = sb.tile([P, 2 * C], F32, tag="slo")
            nc.vector.tensor_tensor_scan(slo, d0v, d1v, BIG, OP.subtract, OP.max)
            shi = sb.tile([P, 2 * C], F32, tag="shi")
            nc.vector.tensor_tensor_scan(shi, d0v, d1v, -BIG, OP.subtract, OP.max)
            rcos = sb.tile([P, C], F32, tag="rcos")
            nc.vector.reciprocal(rcos, cos_d)
            # ntanl = -tan(delta)*dt/L; the sign cancels against -w from the
            # v-scan's sign-alternating output, so w never needs negating.
            ntanl = sb.tile([P, C], F32, tag="ntanl")
            nc.vector.tensor_tensor(ntanl, sin_d, rcos, OP.mult)
            nc.vector.tensor_scalar_mul(ntanl, ntanl, -dt_f / L)

            # ---- x0-derived per-partition biases (off critical path) ----
            nc.vector.tensor_scalar(xrow[0:1, 4:5], xrow[0:1, 3:4],
                                    0.0, 30.0, OP.max, OP.min)      # e0
            nc.vector.tensor_scalar_mul(xrow[0:1, 5:6], xrow[0:1, 4:5], -1.0)
            ne0 = xrow[0:1, 5:6]
            ones_row = sb.tile([1, P], F32, tag="ones_row")
            nc.gpsimd.memset(ones_row, 1.0)
            xb_ps = ps.tile([P, 4], F32, tag="xb")
            nc.tensor.matmul(xb_ps, ones_row, xrow[0:1, 0:4], start=True, stop=True)
            xbs = sb.tile([P, 4], F32, tag="xbs")
            nc.vector.tensor_copy(xbs, xb_ps)
            x00, y00, th0 = xbs[:, 0:1], xbs[:, 1:2], xbs[:, 2:3]

            # ---- other constants: memsets/iota on GpSimd, compares DVE ----
            kmj = sb.tile([P, P], mybir.dt.int32, tag="kmj")   # k - m
            nc.gpsimd.iota(kmj, [[-1, P]], base=0, channel_multiplier=1)
            tri_t = sb.tile([P, P], F32, tag="tri")     # tri[k,m]=1 iff k<m
            nc.vector.tensor_scalar(tri_t, kmj, 0, None, OP.is_lt)
            eye_t = sb.tile([P, P], F32, tag="eye")
            nc.vector.tensor_scalar(eye_t, kmj, 0, None, OP.is_equal)
            one_t = sb.tile([1, 1], F32, tag="one_t")
            nc.gpsimd.memset(one_t, 1.0)
            # Compose scan: each chunk map clip(v+s, lo, hi) is applied in
            # TWO sign-alternating micro-steps:
            #   k0: state = max(s_p - state, lo_p)     # -e -> max(e+s, lo)
            #   k1: state = max(0 - state, -hi_p)      # -> -min(., hi) = -e'
            # 129 chunk slots; slot 0 is a virtual identity map (s=0,
            # lo=-BIG, hi=+BIG == the memset defaults), so -e_p lands at
            # 2p+1 for p=0..127 and feeds the transpose-back matmul directly.
            NCH = 2 * (P + 1)
            d0c = sb.tile([1, NCH], F32, tag="d0c")
            nc.gpsimd.memset(d0c, 0.0)
            d1c = sb.tile([1, NCH], F32, tag="d1c")
            nc.gpsimd.memset(d1c, -BIG)

            # lhsT columns spaced 32 apart so the transposed rows land on
            # quad-aligned PSUM partitions (engines can't start mid-quad).
            stage65 = sb.tile([P, 65], F32, tag="stage65")
            nc.vector.memset(stage65, 0.0)
            nc.vector.tensor_reduce(stage65[:, 0:1], d0v[:, 0:2 * C:2],
                                    mybir.AxisListType.X, OP.add)   # s_p
            nc.scalar.activation(stage65[:, 32:33], shi[:, 2 * C - 1:2 * C], AF.Copy)
            nc.scalar.activation(stage65[:, 64:65], slo[:, 2 * C - 1:2 * C], AF.Copy)
            rows = ps.tile([65, P], F32, tag="rows")  # p0: s_p, p32: -hi_p, p64: -lo_p
            nc.tensor.matmul(rows, stage65[:, 0:65], eye_t, start=True, stop=True)

            # ---- compose scan over the 128 chunk maps ----
            # d0 = [s_p, 0] interleave; d1 = [+lo_p, -hi_p] interleave.
            nc.scalar.activation(d1c[0:1, 2:NCH:2], rows[64:65, :],
                                 AF.Copy, scale=-1.0)           # +lo_p
            nc.vector.tensor_copy(d0c[0:1, 2:NCH:2], rows[0:1, :])
            nc.scalar.activation(d1c[0:1, 3:NCH:2], rows[32:33, :],
                                 AF.Copy)                       # -hi_p
            comp = sb.tile([1, NCH], F32, tag="comp")
            nc.vector.tensor_tensor_scan(comp, d0c, d1c, ne0,
                                         OP.subtract, OP.max)
            nec = ps.tile([P, 1], F32, tag="nec")        # -e_p per partition
            nc.tensor.matmul(nec, comp[0:1, 1:2 * P:2], one_t,
                             start=True, stop=True)

            # ---- v-scan pass 2 + w_in/w_out ----
            sv = sb.tile([P, 2 * C], F32, tag="sv")
            nc.vector.tensor_tensor_scan(sv, d0v, d1v, nec[:, 0:1],
                                         OP.subtract, OP.max)
            OUT = sb.tile([P, 4 * C], F32, tag="OUT")
            nc.scalar.activation(OUT[:, 3:4 * C:4], sv[:, 1:2 * C:2],
                                 AF.Copy, scale=-1.0)   # w_{t+1}

            # ---- theta: prefix sum of g = w*tan(delta)*dt/L ----
            # w_t = -sv[odd] (and -nec for each chunk head); the sign folds
            # into ntanl, so g is built straight from the scan output.
            g = sb.tile([P, C], F32, tag="g")
            nc.vector.tensor_tensor(g[:, 1:C], sv[:, 1:2 * C - 2:2],
                                    ntanl[:, 1:C], OP.mult)
            nc.vector.tensor_tensor(g[:, 0:1], nec[:, 0:1], ntanl[:, 0:1],
                                    OP.mult)
            sg = sb.tile([P, C], F32, tag="sg")
            nc.vector.tensor_tensor_scan(sg, g, g, 0.0, OP.add, OP.bypass)
            offg = ps.tile([P, 1], F32, tag="offg")
            nc.tensor.matmul(offg, tri_t, sg[:, C - 1:C], start=True, stop=True)
            # texc (exclusive scan) overlaps the offset matmul, so th_in is
            # one ts op after the matmul instead of two.
            texc = sb.tile([P, C], F32, tag="texc")
            nc.vector.tensor_tensor(texc, sg, g, OP.subtract)
            th_in = sb.tile([P, C], F32, tag="th_in")    # theta_t
            nc.vector.tensor_scalar(th_in, texc, offg[:, 0:1], th0, OP.add, OP.add)

            # ---- range-reduce theta and theta+pi/2, then ONE Sin over both
            # halves of a shared [P, 2C] tile ----
            trx = sb.tile([P, 2 * C], F32, tag="trx")
            q1 = sb.tile([P, C], F32, tag="q1")
            nc.vector.tensor_scalar(q1, th_in, INV2PI, MAGIC, OP.mult, OP.add)
            n1 = sb.tile([P, C], F32, tag="n1")
            nc.vector.tensor_scalar(n1, q1, MAGIC, TWOPI, OP.subtract, OP.mult)
            nc.vector.tensor_tensor(trx[:, 0:C], th_in, n1, OP.subtract)
            phi = sb.tile([P, C], F32, tag="phi")
            nc.vector.tensor_scalar_add(phi, th_in, HPI)
            q2 = sb.tile([P, C], F32, tag="q2")
            nc.vector.tensor_scalar(q2, phi, INV2PI, MAGIC, OP.mult, OP.add)
            n2 = sb.tile([P, C], F32, tag="n2")
            nc.vector.tensor_scalar(n2, q2, MAGIC, TWOPI, OP.subtract, OP.mult)
            nc.vector.tensor_tensor(trx[:, C:2 * C], phi, n2, OP.subtract)
            tho = sb.tile([P, C], F32, tag="tho")        # theta_{t+1}
            nc.vector.tensor_scalar(tho, sg, offg[:, 0:1], th0, OP.add, OP.add)
            nc.scalar.activation(OUT[:, 2:4 * C:4], tho, AF.Copy)
            sc = sb.tile([P, 2 * C], F32, tag="sc")
            nc.scalar.activation(sc, trx, AF.Sin, bias=zero_b)
            sin_t = sc[:, 0:C]
            cos_t = sc[:, C:2 * C]

            # ---- positions: prefix sums of w*dt*cos/sin(theta) ----
            w_dt = sb.tile([P, C], F32, tag="w_dt")
            nc.scalar.activation(w_dt[:, 1:C], sv[:, 1:2 * C - 2:2],
                                 AF.Copy, scale=-dt_f)
            nc.vector.tensor_scalar_mul(w_dt[:, 0:1], nec, -dt_f)
            c = sb.tile([P, C], F32, tag="c")
            nc.vector.tensor_tensor(c, w_dt, cos_t, OP.mult)
            d = sb.tile([P, C], F32, tag="d")
            nc.vector.tensor_tensor(d, w_dt, sin_t, OP.mult)
            scn = sb.tile([P, C], F32, tag="scn")
            nc.vector.tensor_tensor_scan(scn, c, c, 0.0, OP.add, OP.bypass)
            sdn = sb.tile([P, C], F32, tag="sdn")
            nc.vector.tensor_tensor_scan(sdn, d, d, 0.0, OP.add, OP.bypass)
            stage2 = sb.tile([P, 2], F32, tag="stage2")
            nc.vector.tensor_copy(stage2[:, 0:1], scn[:, C - 1:C])
            nc.vector.tensor_copy(stage2[:, 1:2], sdn[:, C - 1:C])
            offcd = ps.tile([P, 2], F32, tag="offcd")
            nc.tensor.matmul(offcd, tri_t, stage2, start=True, stop=True)
            # px on DVE; py on ACT (Identity is in the same loaded table set
            # as Sin/Copy) so the two final adds run in parallel.
            boffd = sb.tile([P, 1], F32, tag="boffd")
            nc.vector.tensor_scalar(boffd, offcd[:, 1:2], y00, None, OP.add)
            nc.vector.tensor_scalar(OUT[:, 0:4 * C:4], scn, offcd[:, 0:1], x00,
                                    OP.add, OP.add)
            nc.scalar.activation(OUT[:, 1:4 * C:4], sdn, AF.Identity,
                                 bias=boffd)

            # ---- stores ----
            nc.sync.dma_start(
                out=out_d[1:H + 1, :].rearrange("(p j) c -> p (j c)", p=P),
                in_=OUT)
            nc.sync.dma_start(out=out_d[0:1, 0:4], in_=xrow[0:1, 0:4])

    nc.compile()
    return nc


def kernel(x0, U, dt):
    key = float(np.asarray(dt, np.float32).reshape(())[()])
    if key not in _CACHE:
        _CACHE[key] = _build(key)
    nc = _CACHE[key]

    in_map = {
        "x0": np.ascontiguousarray(np.asarray(x0, np.float32)),
        "U": np.ascontiguousarray(np.asarray(U, np.float32)),
    }
    in_maps = [in_map for _ in range(N_CORES)]

    trace = os.environ.get("KB_TRACE", "0") == "1"
    res = run_bass_kernel_spmd(nc, in_maps, list(range(N_CORES)), trace=trace)

    LAST_RUN_INFO.clear()
    LAST_RUN_INFO["exec_time_ns"] = res.exec_time_ns
    if res.instructions_and_trace is not None:
        LAST_RUN_INFO["trace_path"] = res.instructions_and_trace[1]

    return np.asarray(res.results[0]["out"], np.float32).reshape(H + 1, 4)


# revision 41
# speedup vs baseline: 1.0198x; 1.0198x over previous
"""Trainium2 Bass kernel for the KinematicBicycle rollout (H=8192).

kernel(x0, U, dt) -> [8193, 4] float32 trajectory, computed on TRN2.

Algorithm (validated against the jax reference):
  b_t = clip(U[t,0],±3)*dt. The speed recurrence
      w_{t+1} = min(max(w_t + b_t, 0), 30)
  is an alternating clamp chain, computed EXACTLY with the DVE
  tensor_tensor_scan primitive (state = (data0 op0 state) op1 data1) using
  op0=subtract / op1=max over sign-alternating interleaved data:
      even step: state = max(b_t - state, 0)    # state_in = -w_t
      odd  step: state = max(0 - state, -30)    # = -min(., 30) = -w_{t+1}
  Time is chunked [128 partitions x 64 steps]. Chunk-entry speeds come from
  composing per-chunk clamp-add maps F_p(v)=min(max(v+s_p,lo_p),hi_p):
  lo_p/hi_p are obtained by probing each chunk scan with -/+BIG initials, and
  a 2-micro-step composition scan (258 elements on one partition) chains the
  128 maps. Given w, theta is a plain prefix sum of w_t*tan(delta_t)*dt/L and
  x/y are prefix sums of w_t*cos/sin(theta_t)*dt: per-partition
  tensor_tensor_scan + strict-lower-triangular matmul for cross-chunk
  offsets. ACT has only Sin (accurate on ~[-pi,pi]): cos(x)=sin(x+pi/2),
  tan(d)=sin(d)/cos(d); theta is range-reduced with the fp32 magic-constant
  round trick before the table lookups.

The rollout is a single sequential recurrence (no batch dim), so there is
nothing to shard across cores; the program is replicated SPMD on all 8
cores and core 0's output is returned.
"""
import os
import numpy as np

import concourse.bacc as bacc
import concourse.bass as bass
import concourse.mybir as mybir
import concourse.tile as tile
from concourse.bass_utils import run_bass_kernel_spmd

F32 = mybir.dt.float32
OP = mybir.AluOpType
AF = mybir.ActivationFunctionType

H, P, C = 8192, 128, 64
L = 2.7
BIG = 1e30
HPI = float(np.pi / 2)
MAGIC = 12582912.0          # 1.5*2^23: fp32 round-to-nearest via add/sub
INV2PI = float(1.0 / (2.0 * np.pi))
TWOPI = float(2.0 * np.pi)
N_CORES = int(os.environ.get("KB_CORES", "8"))

LAST_RUN_INFO = {}
_CACHE = {}


def _build(dt_val):
    nc = bacc.Bacc("TRN2", target_bir_lowering=False, debug=False)

    dt_f = float(dt_val)

    x0_d = nc.dram_tensor("x0", [4], F32, kind="ExternalInput")
    U_d = nc.dram_tensor("U", [H, 2], F32, kind="ExternalInput")
    out_d = nc.dram_tensor("out", [H + 1, 4], F32, kind="ExternalOutput")

    with tile.TileContext(nc) as tc:
        with (
            tc.tile_pool(name="sb", bufs=1) as sb,
            tc.tile_pool(name="ps", bufs=1, space="PSUM") as ps,
        ):
            # ---- loads; dt is baked into the program as an immediate ----
            Ut = sb.tile([P, 2 * C], F32, tag="Ut")
            nc.sync.dma_start(out=Ut, in_=U_d[:].rearrange("(p j) c -> p (j c)", p=P))
            xrow = sb.tile([1, 8], F32, tag="xrow")
            nc.sync.dma_start(out=xrow[0:1, 0:4],
                              in_=x0_d[:].rearrange("(o a) -> o a", o=1))

            hpi_b = sb.tile([P, 1], F32, tag="hpi_b")
            nc.gpsimd.memset(hpi_b, HPI)
            zero_b = sb.tile([P, 1], F32, tag="zero_b")
            nc.gpsimd.memset(zero_b, 0.0)
            d0v = sb.tile([P, 2 * C], F32, tag="d0v")
            nc.gpsimd.memset(d0v, 0.0)
            d1v = sb.tile([P, 2 * C], F32, tag="d1v")
            nc.gpsimd.memset(d1v, 0.0)
            nc.gpsimd.memset(d1v[:, 1:2 * C:2], -30.0)

            # First ACT op depends only on the early memsets, so the 1.3us
            # ACT table load (inserted before the first ACTIVATE and after
            # its semaphore wait) runs during the DMA window.
            warm = sb.tile([P, 1], F32, tag="warm")
            nc.scalar.activation(warm, hpi_b, AF.Sin, bias=zero_b)

            # ---- controls ----
            dcl = sb.tile([P, C], F32, tag="dcl")
            nc.vector.tensor_scalar(dcl, Ut[:, 1:2 * C:2], -0.6, 0.6, OP.max, OP.min)
            sin_d = sb.tile([P, C], F32, tag="sin_d")
            nc.scalar.activation(sin_d, dcl, AF.Sin, bias=zero_b)
            cos_d = sb.tile([P, C], F32, tag="cos_d")
            nc.scalar.activation(cos_d, dcl, AF.Sin, bias=hpi_b)
            bcl = sb.tile([P, C], F32, tag="bcl")
            nc.vector.tensor_scalar(bcl, Ut[:, 0:2 * C:2], -3.0, 3.0, OP.max, OP.min)
            # write b = bcl*dt straight into the even slots of the scan input
            nc.vector.tensor_scalar_mul(d0v[:, 0:2 * C:2], bcl, dt_f)
            # ---- v-scan pass 1: per-chunk probes; batch-transpose maps ----
            slo = sb.tile([P, 2 * C], F32, tag="slo")
            nc.vector.tensor_tensor_scan(slo, d0v, d1v, BIG, OP.subtract, OP.max)
            shi = sb.tile([P, 2 * C], F32, tag="shi")
            nc.vector.tensor_tensor_scan(shi, d0v, d1v, -BIG, OP.subtract, OP.max)
            rcos = sb.tile([P, C], F32, tag="rcos")
            nc.vector.reciprocal(rcos, cos_d)
            # ntanl = -tan(delta)*dt/L; the sign cancels against -w from the
            # v-scan's sign-alternating output, so w never needs negating.
            ntanl = sb.tile([P, C], F32, tag="ntanl")
            nc.vector.tensor_tensor(ntanl, sin_d, rcos, OP.mult)
            nc.vector.tensor_scalar_mul(ntanl, ntanl, -dt_f / L)

            # ---- x0-derived per-partition biases (off critical path) ----
            nc.vector.tensor_scalar(xrow[0:1, 4:5], xrow[0:1, 3:4],
                                    0.0, 30.0, OP.max, OP.min)      # e0
            nc.vector.tensor_scalar_mul(xrow[0:1, 5:6], xrow[0:1, 4:5], -1.0)
            ne0 = xrow[0:1, 5:6]
            ones_row = sb.tile([1, P], F32, tag="ones_row")
            nc.gpsimd.memset(ones_row, 1.0)
            xb_ps = ps.tile([P, 4], F32, tag="xb")
            nc.tensor.matmul(xb_ps, ones_row, xrow[0:1, 0:4], start=True, stop=True)
            xbs = sb.tile([P, 4], F32, tag="xbs")
            nc.vector.tensor_copy(xbs, xb_ps)
            x00, y00, th0 = xbs[:, 0:1], xbs[:, 1:2], xbs[:, 2:3]

            # ---- other constants: memsets/iota on GpSimd, compares DVE ----
            kmj = sb.tile([P, P], mybir.dt.int32, tag="kmj")   # k - m
            nc.gpsimd.iota(kmj, [[-1, P]], base=0, channel_multiplier=1)
            tri_t = sb.tile([P, P], F32, tag="tri")     # tri[k,m]=1 iff k<m
            nc.vector.tensor_scalar(tri_t, kmj, 0, None, OP.is_lt)
            eye_t = sb.tile([P, P], F32, tag="eye")
            nc.vector.tensor_scalar(eye_t, kmj, 0, None, OP.is_equal)
            one_t = sb.tile([1, 1], F32, tag="one_t")
            nc.gpsimd.memset(one_t, 1.0)
            # Compose scan: each chunk map clip(v+s, lo, hi) is applied in
            # TWO sign-alternating micro-steps:
            #   k0: state = max(s_p - state, lo_p)     # -e -> max(e+s, lo)
            #   k1: state = max(0 - state, -hi_p)      # -> -min(., hi) = -e'
            # 129 chunk slots; slot 0 is a virtual identity map (s=0,
            # lo=-BIG, hi=+BIG == the memset defaults), so -e_p lands at
            # 2p+1 for p=0..127 and feeds the transpose-back matmul directly.
            NCH = 2 * (P + 1)
            d0c = sb.tile([1, NCH], F32, tag="d0c")
            nc.gpsimd.memset(d0c, 0.0)
            d1c = sb.tile([1, NCH], F32, tag="d1c")
            nc.gpsimd.memset(d1c, -BIG)

            # lhsT columns spaced 32 apart so the transposed rows land on
            # quad-aligned PSUM partitions (engines can't start mid-quad).
            stage65 = sb.tile([P, 65], F32, tag="stage65")
            nc.vector.memset(stage65, 0.0)
            nc.vector.tensor_reduce(stage65[:, 0:1], d0v[:, 0:2 * C:2],
                                    mybir.AxisListType.X, OP.add)   # s_p
            nc.scalar.activation(stage65[:, 32:33], shi[:, 2 * C - 1:2 * C], AF.Copy)
            nc.scalar.activation(stage65[:, 64:65], slo[:, 2 * C - 1:2 * C], AF.Copy)
            rows = ps.tile([65, P], F32, tag="rows")  # p0: s_p, p32: -hi_p, p64: -lo_p
            nc.tensor.matmul(rows, stage65[:, 0:65], eye_t, start=True, stop=True)

            # ---- compose scan over the 128 chunk maps ----
            # d0 = [s_p, 0] interleave; d1 = [+lo_p, -hi_p] interleave.
            nc.scalar.activation(d1c[0:1, 2:NCH:2], rows[64:65, :],
                                 AF.Copy, scale=-1.0)           # +lo_p
            nc.scalar.activation(d1c[0:1, 3:NCH:2], rows[32:33, :],
                                 AF.Copy)                       # -hi_p
            nc.scalar.activation(d0c[0:1, 2:NCH:2], rows[0:1, :], AF.Copy)
            comp = sb.tile([1, NCH], F32, tag="comp")
            nc.vector.tensor_tensor_scan(comp, d0c, d1c, ne0,
                                         OP.subtract, OP.max)
            nec = ps.tile([P, 1], F32, tag="nec")        # -e_p per partition
            nc.tensor.matmul(nec, comp[0:1, 1:2 * P:2], one_t,
                             start=True, stop=True)

            # ---- v-scan pass 2 + w_in/w_out ----
            sv = sb.tile([P, 2 * C], F32, tag="sv")
            nc.vector.tensor_tensor_scan(sv, d0v, d1v, nec[:, 0:1],
                                         OP.subtract, OP.max)
            OUT = sb.tile([P, 4 * C], F32, tag="OUT")
            nc.scalar.activation(OUT[:, 3:4 * C:4], sv[:, 1:2 * C:2],
                                 AF.Copy, scale=-1.0)   # w_{t+1}

            # ---- theta: prefix sum of g = w*tan(delta)*dt/L ----
            # w_t = -sv[odd] (and -nec for each chunk head); the sign folds
            # into ntanl, so g is built straight from the scan output.
            g = sb.tile([P, C], F32, tag="g")
            nc.vector.tensor_tensor(g[:, 1:C], sv[:, 1:2 * C - 2:2],
                                    ntanl[:, 1:C], OP.mult)
            nc.vector.tensor_tensor(g[:, 0:1], nec[:, 0:1], ntanl[:, 0:1],
                                    OP.mult)
            sg = sb.tile([P, C], F32, tag="sg")
            nc.vector.tensor_tensor_scan(sg, g, g, 0.0, OP.add, OP.bypass)
            offg = ps.tile([P, 1], F32, tag="offg")
            nc.tensor.matmul(offg, tri_t, sg[:, C - 1:C], start=True, stop=True)
            # texc (exclusive scan) overlaps the offset matmul, so th_in is
            # one ts op after the matmul instead of two.
            texc = sb.tile([P, C], F32, tag="texc")
            nc.vector.tensor_tensor(texc, sg, g, OP.subtract)
            th_in = sb.tile([P, C], F32, tag="th_in")    # theta_t
            nc.vector.tensor_scalar(th_in, texc, offg[:, 0:1], th0, OP.add, OP.add)

            # ---- range-reduce theta and theta+pi/2, then ONE Sin over both
            # halves of a shared [P, 2C] tile ----
            trx = sb.tile([P, 2 * C], F32, tag="trx")
            q1 = sb.tile([P, C], F32, tag="q1")
            nc.vector.tensor_scalar(q1, th_in, INV2PI, MAGIC, OP.mult, OP.add)
            n1 = sb.tile([P, C], F32, tag="n1")
            nc.vector.tensor_scalar(n1, q1, MAGIC, TWOPI, OP.subtract, OP.mult)
            nc.vector.tensor_tensor(trx[:, 0:C], th_in, n1, OP.subtract)
            phi = sb.tile([P, C], F32, tag="phi")
            nc.vector.tensor_scalar_add(phi, th_in, HPI)
            q2 = sb.tile([P, C], F32, tag="q2")
            nc.vector.tensor_scalar(q2, phi, INV2PI, MAGIC, OP.mult, OP.add)
            n2 = sb.tile([P, C], F32, tag="n2")
            nc.vector.tensor_scalar(n2, q2, MAGIC, TWOPI, OP.subtract, OP.mult)
            nc.vector.tensor_tensor(trx[:, C:2 * C], phi, n2, OP.subtract)
            tho = sb.tile([P, C], F32, tag="tho")        # theta_{t+1}
            nc.vector.tensor_scalar(tho, sg, offg[:, 0:1], th0, OP.add, OP.add)
            nc.scalar.activation(OUT[:, 2:4 * C:4], tho, AF.Copy)
            sc = sb.tile([P, 2 * C], F32, tag="sc")
            nc.scalar.activation(sc, trx, AF.Sin, bias=zero_b)
            sin_t = sc[:, 0:C]
            cos_t = sc[:, C:2 * C]

            # ---- positions: prefix sums of w*dt*cos/sin(theta) ----
            w_dt = sb.tile([P, C], F32, tag="w_dt")
            nc.scalar.activation(w_dt[:, 1:C], sv[:, 1:2 * C - 2:2],
                                 AF.Copy, scale=-dt_f)
            nc.vector.tensor_scalar_mul(w_dt[:, 0:1], nec, -dt_f)
            c = sb.tile([P, C], F32, tag="c")
            nc.vector.tensor_tensor(c, w_dt, cos_t, OP.mult)
            d = sb.tile([P, C], F32, tag="d")
            nc.vector.tensor_tensor(d, w_dt, sin_t, OP.mult)
            scn = sb.tile([P, C], F32, tag="scn")
            nc.vector.tensor_tensor_scan(scn, c, c, 0.0, OP.add, OP.bypass)
            sdn = sb.tile([P, C], F32, tag="sdn")
            nc.vector.tensor_tensor_scan(sdn, d, d, 0.0, OP.add, OP.bypass)
            stage2 = sb.tile([P, 2], F32, tag="stage2")
            nc.vector.tensor_copy(stage2[:, 0:1], scn[:, C - 1:C])
            nc.vector.tensor_copy(stage2[:, 1:2], sdn[:, C - 1:C])
            offcd = ps.tile([P, 2], F32, tag="offcd")
            nc.tensor.matmul(offcd, tri_t, stage2, start=True, stop=True)
            # px on DVE; py on ACT (Identity is in the same loaded table set
            # as Sin/Copy) so the two final adds run in parallel.
            boffd = sb.tile([P, 1], F32, tag="boffd")
            nc.vector.tensor_scalar(boffd, offcd[:, 1:2], y00, None, OP.add)
            nc.vector.tensor_scalar(OUT[:, 0:4 * C:4], scn, offcd[:, 0:1], x00,
                                    OP.add, OP.add)
            nc.scalar.activation(OUT[:, 1:4 * C:4], sdn, AF.Identity,
                                 bias=boffd)

            # ---- stores ----
            nc.sync.dma_start(
                out=out_d[1:H + 1, :].rearrange("(p j) c -> p (j c)", p=P),
                in_=OUT)
            nc.sync.dma_start(out=out_d[0:1, 0:4], in_=xrow[0:1, 0:4])

    nc.compile()
    return nc


def kernel(x0, U, dt):
    key = float(np.asarray(dt, np.float32).reshape(())[()])
    if key not in _CACHE:
        _CACHE[key] = _build(key)
    nc = _CACHE[key]

    in_map = {
        "x0": np.ascontiguousarray(np.asarray(x0, np.float32)),
        "U": np.ascontiguousarray(np.asarray(U, np.float32)),
    }
    in_maps = [in_map for _ in range(N_CORES)]

    trace = os.environ.get("KB_TRACE", "0") == "1"
    res = run_bass_kernel_spmd(nc, in_maps, list(range(N_CORES)), trace=trace)

    LAST_RUN_INFO.clear()
    LAST_RUN_INFO["exec_time_ns"] = res.exec_time_ns
    if res.instructions_and_trace is not None:
        LAST_RUN_INFO["trace_path"] = res.instructions_and_trace[1]

    return np.asarray(res.results[0]["out"], np.float32).reshape(H + 1, 4)
